# revision 37
# baseline (speedup 1.0000x reference)
"""Trainium2 Bass kernel for nn_BCCLayer (bilinear co-attention + pooling + batchnorm).

Math
----
Per batch b and map direction, with P the softmax-axis side and R the
other side:
  G[u,q] = (relu(P@Wq^T+Qb)*h) . relu(R@Wk^T+Kb)[q]      (u,q in [0,2000))
  S_all[q] = sum_u exp(G[u,q]),  S_w[q] = sum_u mask_p[u]*exp(G[u,q])
  w[q] = mask_v[q]/L * S_w[q]/S_all[q]
  contrib[k] = sum_q w[q] * relu(R@Wk^T+Kb)[q,k]
(per-column shifts of G, incl. h_bias, cancel in S_w/S_all; |G| small so
exp needs no max-subtraction). Host does the [4,512] batchnorm epilogue.

Sharding: 8 independent (batch, map) units -> one per NeuronCore, SPMD.

Engine split (the kernel is exp-bound; ACT is the only exp engine):
  ACT  : exp of G (paired u-tiles, [128,2,512] per instr) + first vt chunk
  DVE  : all FC evacuations as direct fp8/f32r writes from PSUM
  PE   : fp8 DoubleRow FCs + G + S-reduction, bf16 value-chain FC, w/contrib
  Pool : small SBUF copies + output DMA (and mixed-sign-chunk h multiply)

h_mat folding: k-indices are host-permuted so sign(h) is constant per
128-k chunk; |h| (and the sign, via negated weights + a min-with-0
instead of max-with-0) folds into Wq/Qb, so ut8 = h*relu(fc) comes out
of a single 2-op DVE tensor_scalar straight to fp8. A chunk with mixed
signs falls back to bf16 staging + a Pool multiply by h.

Value chain (feeds batchnorm whose tiny across-batch variance amplifies
errors ~40x) runs in bf16 via an XBAR DMA-transpose of R; the q window
is trimmed to ceil(max_valid/128) 128-col tiles (valid cols permuted to
the front per core, padded cols have mask_v = 0 so garbage is masked).

Scales: fp8 weights carry 64x (their ~3e-3 entries would be subnormal);
exp applies the 1/64^2 correction, the value chain carries 64x through
vnat and folds 1/64 into the w column scale.
"""

import numpy as np

L = 2000
LP = 2048  # L padded to a multiple of 512
HD = 256
KD = 512
B = 4
EPS = 1e-5
NCORES = 8
WSCALE = 64.0   # fp8 weight scale
NLT = LP // 128  # 16 u tiles

_NC_CACHE = {}


def _build_nc(nqt=13, modes=("max", "max", "min", "min")):
    import concourse.mybir as mybir
    import concourse.tile as tile
    from concourse import bacc

    f32 = mybir.dt.float32
    bf16 = mybir.dt.bfloat16
    fp8 = mybir.dt.float8e4
    f32r = mybir.dt.float32r
    u16 = mybir.dt.uint16
    AF = mybir.ActivationFunctionType
    DR = mybir.MatmulPerfMode.DoubleRow
    ALU = mybir.AluOpType

    NQT = nqt
    NQP = 128 * NQT
    # q chunks of <=512 for the vt FC and the G/exp spans
    spans = []
    c0 = 0
    while c0 < NQP:
        spans.append((c0, min(512, NQP - c0)))
        c0 += 512
    NSP = len(spans)
    NHC = HD // 128   # 2 h chunks
    NKC = KD // 128   # 4 k chunks

    nc = bacc.Bacc("TRN2", target_bir_lowering=False)

    # all sequence tensors arrive host-pretransposed: partition dim first
    p8_in = nc.dram_tensor("p8_in", [HD // 2, LP], u16, kind="ExternalInput")
    r8_in = nc.dram_tensor("r8_in", [HD // 2, NQP], u16, kind="ExternalInput")
    # bf16 R^T + 64*Wk^T h-chunks for the value chain (bf16 keeps the
    # fixed-pattern weight-quantization error out of the batchnorm)
    rbf_in = nc.dram_tensor("rbf_in", [128, NHC, NQP], bf16, kind="ExternalInput")
    wkbf_in = nc.dram_tensor("wkbf_in", [128, NHC, KD], bf16, kind="ExternalInput")
    kbd_in = nc.dram_tensor("kbd_in", [KD], bf16, kind="ExternalInput")  # 64*Kb/128
    # 64*W^T in (c, s, k) h-pair interleaved order, fp8 (|h|/sign folded into wq8)
    wq8_in = nc.dram_tensor("wq8_in", [128, 2, KD], fp8, kind="ExternalInput")
    wk8_in = nc.dram_tensor("wk8_in", [128, 2, KD], fp8, kind="ExternalInput")
    # cols 0-3 ut bias (64*|h|*Qb signed), 4-7 64*Kb, 8-11 h (mix chunks),
    # cols 12-13 = eye(2) for the tiny S transposes
    bias_cols = nc.dram_tensor("bias_cols", [128, 14], f32, kind="ExternalInput")
    # cols 0-15 mask_p {0,1}; 16-31 valid {0,1}; 32.. mask_v/(64L) packed
    mask_cols = nc.dram_tensor("mask_cols", [128, 32 + NQT], f32, kind="ExternalInput")
    out = nc.dram_tensor("out", [1, KD], f32, kind="ExternalOutput")

    with tile.TileContext(nc) as tc:
        import contextlib
        ctx = contextlib.ExitStack()
        with ctx:
            singles = ctx.enter_context(tc.tile_pool(name="singles", bufs=1))
            stage = ctx.enter_context(tc.tile_pool(name="stage", bufs=2))
            wsmall = ctx.enter_context(tc.tile_pool(name="wsmall", bufs=8))
            epool = ctx.enter_context(tc.tile_pool(name="epool", bufs=4))
            pfc = ctx.enter_context(tc.tile_pool(name="pfc", bufs=2, space="PSUM"))
            pg = ctx.enter_context(tc.tile_pool(name="pg", bufs=2, space="PSUM"))
            ps = ctx.enter_context(tc.tile_pool(name="ps", bufs=1, space="PSUM"))
            pc = ctx.enter_context(tc.tile_pool(name="pc", bufs=1, space="PSUM"))

            # ---- prime ACT (act-table load) and warm up PE immediately ----
            ones_bf = singles.tile([128, 128], bf16)
            nc.vector.memset(ones_bf, 1.0)
            prime = singles.tile([128, 8], f32)
            nc.scalar.copy(prime, ones_bf[:, 0:8])
            warm_ps = pfc.tile([128, 512], f32, tag="fc")
            nc.tensor.matmul(warm_ps[:, 0:128], lhsT=ones_bf, rhs=ones_bf)

            # ---- input DMAs. Everything is host-pretransposed, so these
            # are plain copies. The four small critical params issue from
            # the ACT HWDGE queue first (they carry no DMA-engine-reuse
            # waits, so they don't block ACT's compute stream); the bulk
            # sequence tensors stream on the SP queue. ----
            p8t = singles.tile([128, LP], u16)
            r8t = singles.tile([128, NQP], u16)
            wq8 = singles.tile([128, 2, KD], fp8)
            wk8 = singles.tile([128, 2, KD], fp8)
            bcols = singles.tile([128, 14], f32)
            mcols = singles.tile([128, 32 + NQT], f32)

            # first chunks of p8/r8 go on the SWDGE queue so the FC ladder
            # starts ~3.5us; the rest streams behind them
            nc.gpsimd.dma_start(p8t[:, 0:512], p8_in[:, 0:512])
            nc.gpsimd.dma_start(r8t[:, 0 : min(512, NQP)], r8_in[:, 0 : min(512, NQP)])
            nc.gpsimd.dma_start(p8t[:, 512:LP], p8_in[:, 512:LP])
            if NQP > 512:
                nc.gpsimd.dma_start(r8t[:, 512:NQP], r8_in[:, 512:NQP])
            nc.sync.dma_start(wq8, wq8_in[:])
            nc.sync.dma_start(bcols, bias_cols[:])
            nc.sync.dma_start(wk8, wk8_in[:])
            nc.sync.dma_start(mcols, mask_cols[:])
            wk_bf = singles.tile([128, NHC, KD], bf16)
            nc.sync.dma_start(wk_bf, wkbf_in[:])
            rtb = singles.tile([128, NHC, NQP], bf16)
            nc.sync.dma_start(rtb, rbf_in[:])
            kbd_bc = singles.tile([128, KD], bf16)
            nc.gpsimd.dma_start(kbd_bc, kbd_in[:].partition_broadcast(128))

            qb_col = bcols[:, 0:NKC]                 # signed 64*|h|*Qb (or 64*Qb mix)
            kb64_col = bcols[:, NKC : 2 * NKC]       # 64*Kb
            h_col = bcols[:, 2 * NKC : 3 * NKC]      # h (mix chunks only)
            ident2 = bcols[:, 12:14]                 # eye(2) on partitions 0-1
            mp_col = mcols[:, 0:NLT]
            valid_col = mcols[:, NLT : 2 * NLT]
            mv_col = mcols[:, 2 * NLT :]             # mask_v/(64L), packed

            # reduction stationary, DoubleRow-paired over u-tile pairs (Pool
            # builds it so DVE stays clear for the FC evacuations)
            rbuf8 = singles.tile([128, 2, NLT // 2, 2], fp8)
            for ko in range(2):
                nc.gpsimd.tensor_copy(rbuf8[:, ko, :, 0], valid_col[:, ko::2])
                nc.gpsimd.tensor_copy(rbuf8[:, ko, :, 1], mp_col[:, ko::2])

            # fp8 views with the h-pair as the DoubleRow interleave dim
            p8v = p8t[:].bitcast(fp8).rearrange("p (l two) -> p two l", two=2)
            r8v = r8t[:].bitcast(fp8).rearrange("p (l two) -> p two l", two=2)

            ut8 = singles.tile([128, NKC, LP], fp8)
            vt8 = singles.tile([128, NKC, NQP], fp8)
            vnat = singles.tile([128, NQT, KD], f32r)
            wcol = singles.tile([128, NQT], f32r)
            s_sb = singles.tile([2, NQP], f32)

            # ---- FC phase: ut (Q-side of P, all 2048 u) and vt (K-side of
            # R, valid q only), fp8 DoubleRow, evacuated straight to fp8.
            # vt chunks >=2 are deferred into G-loop slots so the DVE
            # stream reaches the span-0 S evacuation without backlog. ----
            def ut_fc(vc, kc):
                sl = slice(vc * 512, (vc + 1) * 512)
                pm = pfc.tile([128, 512], f32, tag="fc")
                nc.tensor.matmul(
                    pm,
                    lhsT=wq8[:, :, kc * 128 : (kc + 1) * 128],
                    rhs=p8v[:, :, sl],
                    perf_mode=DR,
                )
                if modes[kc] == "mix":
                    sb = stage.tile([128, 512], bf16, tag="st")
                    nc.vector.tensor_scalar(
                        sb, pm, qb_col[:, kc : kc + 1], 0.0, ALU.add, ALU.max
                    )
                    nc.gpsimd.tensor_scalar_mul(
                        ut8[:, kc, sl], sb, h_col[:, kc : kc + 1]
                    )
                else:
                    op1 = ALU.max if modes[kc] == "max" else ALU.min
                    nc.vector.tensor_scalar(
                        ut8[:, kc, sl], pm, qb_col[:, kc : kc + 1], 0.0,
                        ALU.add, op1,
                    )

            def vt_fc(vc, kc):
                qs = slice(vc * 512, min((vc + 1) * 512, NQP))
                pm2 = pfc.tile([128, 512], f32, tag="fc")
                nc.tensor.matmul(
                    pm2[:, 0 : qs.stop - qs.start],
                    lhsT=wk8[:, :, kc * 128 : (kc + 1) * 128],
                    rhs=r8v[:, :, qs],
                    perf_mode=DR,
                )
                if vc == 0:
                    # ACT is otherwise idle before the first exp
                    nc.scalar.activation(
                        vt8[:, kc, qs], pm2[:, 0 : qs.stop - qs.start],
                        AF.Relu, bias=kb64_col[:, kc : kc + 1],
                    )
                else:
                    nc.vector.tensor_scalar(
                        vt8[:, kc, qs], pm2[:, 0 : qs.stop - qs.start],
                        kb64_col[:, kc : kc + 1], 0.0, ALU.add, ALU.max,
                    )

            nqc = (NQP + 511) // 512  # vt chunks
            for kc in range(NKC):
                ut_fc(0, kc)
            inline_vt = 1 if NSP >= 2 else nqc
            for vc in range(min(inline_vt, nqc)):
                for kc in range(NKC):
                    vt_fc(vc, kc)
            for vc in range(1, 4):
                for kc in range(NKC):
                    ut_fc(vc, kc)
            # deferred vt chunk c runs in span c-1's slots (DVE reaches it
            # right after the ut chain, before the span's S evacuation)
            vt_sched = {}
            for c in range(inline_vt, nqc):
                vt_sched.setdefault(min(c - 1, NSP - 1), []).append(c)

            # ---- value chain (bf16), w math, contrib: interleaved into the
            # G loop's slots below ----
            # value chain: same fp8 DoubleRow operands as the vt FC, but
            # transposed layout ([q, k]); bias rides a bf16 ones x Kb/128
            # matmul; the 64x scale is folded into the w column scale
            def fc_nat(qt):
                pm = pfc.tile([128, 512], f32, tag="fc")
                for hc in range(NHC):
                    nc.tensor.matmul(
                        pm,
                        lhsT=rtb[:, hc, qt * 128 : (qt + 1) * 128],
                        rhs=wk_bf[:, hc, :],
                        start=(hc == 0),
                        stop=False,
                    )
                nc.tensor.matmul(
                    pm, lhsT=ones_bf, rhs=kbd_bc[:],
                    start=False, stop=True, skip_group_check=True,
                )
                nc.vector.tensor_scalar_max(vnat[:, qt, :], pm, 0.0)

            def w_math(si):
                q0, w = spans[si]
                nc.vector.tensor_copy(s_sb[:, q0 : q0 + w], s_t[si][:, 0:w])
                for j in range(w // 128):
                    qt = q0 // 128 + j
                    st_ps = pfc.tile([128, 512], f32, tag="fc")
                    nc.tensor.transpose(
                        st_ps[:, 0:2],
                        s_sb[:, qt * 128 : (qt + 1) * 128],
                        ident2[0:2, :],
                    )
                    s2 = wsmall.tile([128, 2], f32, tag="s2")
                    nc.vector.tensor_copy(s2, st_ps[:, 0:2])
                    rcp = wsmall.tile([128, 1], f32, tag="rcp")
                    nc.vector.reciprocal(rcp, s2[:, 0:1])
                    nc.vector.tensor_mul(rcp, rcp, s2[:, 1:2])
                    nc.vector.tensor_mul(
                        wcol[:, qt : qt + 1], rcp, mv_col[:, qt : qt + 1]
                    )

            c_ps = pc.tile([1, KD], f32)
            contrib_n = [0]

            def contrib(qt):
                contrib_n[0] += 1
                nc.tensor.matmul(
                    c_ps,
                    lhsT=wcol[:, qt : qt + 1],
                    rhs=vnat[:, qt, :],
                    start=(contrib_n[0] == 1),
                    stop=(contrib_n[0] == NQT),
                    skip_group_check=True,
                )

            # slot scheduler: fc_nat goes into span>=1 slots; a span's
            # contrib mms are emitted once its w_math ran and its fc_nats
            # are all emitted
            fc_done = [False] * NQT
            wm_done = [False] * NSP
            ct_done = [False] * NSP
            fc_iter = iter(range(NQT))

            def try_contrib():
                for si2, (q0, w) in enumerate(spans):
                    if ct_done[si2] or not wm_done[si2]:
                        continue
                    qts = range(q0 // 128, (q0 + w) // 128)
                    if all(fc_done[qt] for qt in qts):
                        for qt in qts:
                            contrib(qt)
                        ct_done[si2] = True

            # fc_nat spreads over spans >=1 (half rate in span 1, where DVE
            # still carries the deferred vt evacuations)
            def slot(si, ltp):
                pend = vt_sched.get(si)
                if pend:
                    c = pend.pop(0)
                    for kc in range(NKC):
                        vt_fc(c, kc)
                    return
                if si >= 1 and not (si == 1 and ltp % 2 == 0):
                    qt = next(fc_iter, None)
                    if qt is not None:
                        fc_nat(qt)
                        fc_done[qt] = True
                        try_contrib()

            # ---- G + exp + S reduction ----
            s_t = [ps.tile([2, 512], f32, tag="s", name=f"s_ps_{si}") for si in range(NSP)]
            for si, (q0, w) in enumerate(spans):
                for ltp in range(NLT // 2):
                    et = epool.tile([128, 2, 512], fp8, tag="e")
                    gp = pg.tile([128, 2, 512], f32, tag="g")
                    for sub in range(2):
                        lt = 2 * ltp + sub
                        for j in range(2):
                            nc.tensor.matmul(
                                gp[:, sub, 0:w],
                                lhsT=ut8[:, 2 * j : 2 * j + 2, lt * 128 : (lt + 1) * 128],
                                rhs=vt8[:, 2 * j : 2 * j + 2, q0 : q0 + w],
                                start=(j == 0),
                                stop=(j == 1),
                                perf_mode=DR,
                            )
                    nc.scalar.activation(
                        et[:, :, 0:w], gp[:, :, 0:w], AF.Exp,
                        scale=1.0 / (WSCALE * WSCALE),
                    )
                    nc.tensor.matmul(
                        s_t[si][:, 0:w],
                        lhsT=rbuf8[:, :, ltp, :],
                        rhs=et[:, :, 0:w],
                        start=(ltp == 0), stop=(ltp == NLT // 2 - 1),
                        perf_mode=DR,
                        skip_group_check=True,
                    )
                    slot(si, ltp)
                w_math(si)
                wm_done[si] = True
                try_contrib()

            # drain any stragglers (small NQT configs)
            for qt in fc_iter:
                fc_nat(qt)
                fc_done[qt] = True
            try_contrib()
            assert all(ct_done), "contrib scheduling incomplete"

            out_sb = singles.tile([1, KD], f32)
            nc.vector.tensor_copy(out_sb, c_ps)
            nc.sync.dma_start(out[:], out_sb)

    nc.finalize()
    return nc


def _get_nc(nqt=13, modes=("max", "max", "min", "min")):
    key = (nqt, tuple(modes))
    if key not in _NC_CACHE:
        _NC_CACHE[key] = _build_nc(nqt, tuple(modes))
    return _NC_CACHE[key]


def kernel(**inputs) -> np.ndarray:
    import ml_dtypes
    from concourse.bass_utils import run_bass_kernel_spmd

    X = np.asarray(inputs["X"], dtype=np.float32)
    Y = np.asarray(inputs["Y"], dtype=np.float32)
    m1 = np.asarray(inputs["mask1"], dtype=np.float32)
    m2 = np.asarray(inputs["mask2"], dtype=np.float32)
    Qv = np.asarray(inputs["Qv"], dtype=np.float32)
    Qg = np.float32(np.asarray(inputs["Qg"]))
    Qb = np.asarray(inputs["Qb"], dtype=np.float32)
    Kv = np.asarray(inputs["Kv"], dtype=np.float32)
    Kg = np.float32(np.asarray(inputs["Kg"]))
    Kb = np.asarray(inputs["Kb"], dtype=np.float32)
    hm = np.asarray(inputs["h_mat"], dtype=np.float32)
    gamma = np.asarray(inputs["gamma"], dtype=np.float32)
    beta = np.asarray(inputs["beta"], dtype=np.float32)

    Wq = (Qg / np.float32(np.linalg.norm(Qv))) * Qv  # [KD, HD]
    Wk = (Kg / np.float32(np.linalg.norm(Kv))) * Kv

    # permute k so sign(h) is constant per 128-chunk where possible
    perm_k = np.argsort(hm < 0, kind="stable")
    hmp = hm[perm_k]
    Wqp, Qbp = Wq[perm_k], Qb[perm_k]
    Wkp, Kbp = Wk[perm_k], Kb[perm_k]
    modes = []
    wq_rows = np.empty_like(Wqp)
    qb_eff = np.empty_like(Qbp)
    for kc in range(4):
        slk = slice(kc * 128, (kc + 1) * 128)
        hs = hmp[slk]
        if np.all(hs >= 0):
            modes.append("max")
            sc = hs
        elif np.all(hs < 0):
            modes.append("min")
            sc = hs  # negative scale = |h| fold + weight negation in one
        else:
            modes.append("mix")
            sc = np.ones_like(hs)
        wq_rows[slk] = sc[:, None] * Wqp[slk]
        qb_eff[slk] = sc * Qbp[slk]
    modes = tuple(modes)

    # 64*W^T with rows pairing consecutive h for the DoubleRow interleave
    wq8_in = np.ascontiguousarray(
        (WSCALE * wq_rows.T).reshape(128, 2, KD).astype(ml_dtypes.float8_e4m3)
    )
    wk8_in = np.ascontiguousarray(
        (WSCALE * Wkp.T).reshape(128, 2, KD).astype(ml_dtypes.float8_e4m3)
    )
    ey = np.zeros((128, 2), np.float32)
    ey[0, 0] = ey[1, 1] = 1.0
    bias_cols = np.ascontiguousarray(
        np.concatenate(
            [
                (WSCALE * qb_eff).reshape(4, 128).T,
                (WSCALE * Kbp).reshape(4, 128).T,
                hmp.reshape(4, 128).T,
                ey,
            ],
            axis=1,
        )
    ).astype(np.float32)  # [128, 14]
    wkbf_in = np.ascontiguousarray(
        (WSCALE * Wkp.T).reshape(2, 128, KD).transpose(1, 0, 2)
    ).astype(ml_dtypes.bfloat16)
    kbd = (WSCALE * Kbp / 128.0).astype(ml_dtypes.bfloat16)

    def padded(v2000):
        p = np.zeros((LP,), np.float32)
        p[:L] = v2000
        return p.reshape(16, 128)

    valid = padded(np.ones(L, np.float32))

    def pad_seq(s):
        p = np.zeros((LP, HD), np.float32)
        p[:L] = s
        return p

    # Only q columns with mask_v > 0 contribute; permute them to the front
    # and trim the computed q window to 128-col tiles covering the max
    # valid count across cores.
    units = []
    max_nv = 0
    for b in range(B):
        for m in range(2):
            if m == 0:
                P, R, mp, mv = X[b], Y[b], m1[b], m2[b]
            else:
                P, R, mp, mv = Y[b], X[b], m2[b], m1[b]
            perm = np.argsort(mv <= 0, kind="stable")
            max_nv = max(max_nv, int((mv > 0).sum()))
            units.append((P, R, mp, mv, perm))
    nqt = max(1, -(-max_nv // 128))
    NQP = 128 * nqt

    in_maps = []
    for P, R, mp, mv, perm in units:
        nperm = min(NQP, L)
        Rp = np.zeros((NQP, HD), np.float32)
        Rp[:nperm] = R[perm[:nperm]]
        mvp = np.zeros((NQP,), np.float32)
        mvp[:nperm] = mv[perm[:nperm]] * (1.0 / (L * WSCALE))
        mask_cols = np.ascontiguousarray(
            np.concatenate(
                [padded(mp), valid, mvp.reshape(nqt, 128)], axis=0
            ).T
        ).astype(np.float32)  # [128, 32 + nqt]
        p8 = np.ascontiguousarray(
            pad_seq(P).astype(ml_dtypes.float8_e4m3).view(np.uint16).T
        )
        r8 = np.ascontiguousarray(
            Rp.astype(ml_dtypes.float8_e4m3).view(np.uint16).T
        )
        rbf = np.ascontiguousarray(
            Rp.astype(ml_dtypes.bfloat16).T.reshape(2, 128, NQP).transpose(1, 0, 2)
        )
        in_maps.append(
            {
                "p8_in": p8,
                "r8_in": r8,
                "rbf_in": rbf,
                "wq8_in": wq8_in,
                "wk8_in": wk8_in,
                "bias_cols": bias_cols,
                "mask_cols": mask_cols,
                "wkbf_in": wkbf_in,
                "kbd_in": kbd,
            }
        )

    nc = _get_nc(nqt, modes)
    res = run_bass_kernel_spmd(nc, in_maps, core_ids=list(range(NCORES)))
    contribs = np.stack([r["out"][0] for r in res.results]).astype(np.float64)

    pooled = contribs[0::2] + contribs[1::2]  # [B, KD] in permuted-k order
    mu = pooled.mean(axis=0)
    var = pooled.var(axis=0)
    outv = gamma[perm_k] * (pooled - mu) / np.sqrt(var + EPS) + beta[perm_k]
    inv = np.empty_like(perm_k)
    inv[perm_k] = np.arange(KD)
    return outv[:, inv].astype(np.float32)


# revision 42
# speedup vs baseline: 1.0659x; 1.0659x over previous
"""Trainium2 Bass kernel for nn_BCCLayer (bilinear co-attention + pooling + batchnorm).

Math
----
Per batch b and map direction, with P the softmax-axis side and R the
other side:
  G[u,q] = (relu(P@Wq^T+Qb)*h) . relu(R@Wk^T+Kb)[q]      (u,q in [0,2000))
  S_all[q] = sum_u exp(G[u,q]),  S_w[q] = sum_u mask_p[u]*exp(G[u,q])
  w[q] = mask_v[q]/L * S_w[q]/S_all[q]
  contrib[k] = sum_q w[q] * relu(R@Wk^T+Kb)[q,k]
(per-column shifts of G, incl. h_bias, cancel in S_w/S_all; |G| small so
exp needs no max-subtraction). Host does the [4,512] batchnorm epilogue.

Sharding: 8 independent (batch, map) units -> one per NeuronCore, SPMD.

Engine split (the kernel is exp-bound; ACT is the only exp engine):
  ACT  : exp of G (paired u-tiles, [128,2,512] per instr) + first vt chunk
  DVE  : all FC evacuations as direct fp8/f32r writes from PSUM
  PE   : fp8 DoubleRow FCs + G + S-reduction, bf16 value-chain FC, w/contrib
  Pool : small SBUF copies + output DMA (and mixed-sign-chunk h multiply)

h_mat folding: k-indices are host-permuted so sign(h) is constant per
128-k chunk; |h| (and the sign, via negated weights + a min-with-0
instead of max-with-0) folds into Wq/Qb, so ut8 = h*relu(fc) comes out
of a single 2-op DVE tensor_scalar straight to fp8. A chunk with mixed
signs falls back to bf16 staging + a Pool multiply by h.

Value chain (feeds batchnorm whose tiny across-batch variance amplifies
errors ~40x) runs in bf16 via an XBAR DMA-transpose of R; the q window
is trimmed to ceil(max_valid/128) 128-col tiles (valid cols permuted to
the front per core, padded cols have mask_v = 0 so garbage is masked).

Scales: fp8 weights carry 64x (their ~3e-3 entries would be subnormal);
exp applies the 1/64^2 correction, the value chain carries 64x through
vnat and folds 1/64 into the w column scale.
"""

import numpy as np

L = 2000
LP = 2048  # L padded to a multiple of 512
HD = 256
KD = 512
B = 4
EPS = 1e-5
NCORES = 8
WSCALE = 64.0   # fp8 weight scale
NLT = LP // 128  # 16 u tiles

_NC_CACHE = {}


def _build_nc(nqt=13, modes=("max", "max", "min", "min")):
    import concourse.mybir as mybir
    import concourse.tile as tile
    from concourse import bacc

    f32 = mybir.dt.float32
    bf16 = mybir.dt.bfloat16
    fp8 = mybir.dt.float8e4
    f32r = mybir.dt.float32r
    u16 = mybir.dt.uint16
    AF = mybir.ActivationFunctionType
    DR = mybir.MatmulPerfMode.DoubleRow
    ALU = mybir.AluOpType

    NQT = nqt
    NQP = 128 * NQT
    # q chunks of <=512 for the vt FC and the G/exp spans
    spans = []
    c0 = 0
    while c0 < NQP:
        spans.append((c0, min(512, NQP - c0)))
        c0 += 512
    NSP = len(spans)
    NHC = HD // 128   # 2 h chunks
    NKC = KD // 128   # 4 k chunks

    nc = bacc.Bacc("TRN2", target_bir_lowering=False)

    # all sequence tensors arrive host-pretransposed: partition dim first
    p8_in = nc.dram_tensor("p8_in", [HD // 2, LP], u16, kind="ExternalInput")
    r8_in = nc.dram_tensor("r8_in", [HD // 2, NQP], u16, kind="ExternalInput")
    # bf16 R^T + 64*Wk^T h-chunks for the value chain (bf16 keeps the
    # fixed-pattern weight-quantization error out of the batchnorm)
    rbf_in = nc.dram_tensor("rbf_in", [128, NHC, NQP], bf16, kind="ExternalInput")
    wkbf_in = nc.dram_tensor("wkbf_in", [128, NHC, KD], bf16, kind="ExternalInput")
    kbd_in = nc.dram_tensor("kbd_in", [KD], bf16, kind="ExternalInput")  # 64*Kb/128
    # 64*W^T in (c, s, k) h-pair interleaved order, fp8 (|h|/sign folded into wq8)
    wq8_in = nc.dram_tensor("wq8_in", [128, 2, KD], fp8, kind="ExternalInput")
    wk8_in = nc.dram_tensor("wk8_in", [128, 2, KD], fp8, kind="ExternalInput")
    # cols 0-3 ut bias (64*|h|*Qb signed), 4-7 64*Kb, 8-11 h (mix chunks),
    # cols 12-13 = eye(2) for the tiny S transposes
    bias_cols = nc.dram_tensor("bias_cols", [128, 14], f32, kind="ExternalInput")
    # cols 0-15 mask_p {0,1}; 16-31 valid {0,1}; 32.. mask_v/(64L) packed
    mask_cols = nc.dram_tensor("mask_cols", [128, 32 + NQT], f32, kind="ExternalInput")
    out = nc.dram_tensor("out", [1, KD], f32, kind="ExternalOutput")

    with tile.TileContext(nc) as tc:
        import contextlib
        ctx = contextlib.ExitStack()
        with ctx:
            singles = ctx.enter_context(tc.tile_pool(name="singles", bufs=1))
            stage = ctx.enter_context(tc.tile_pool(name="stage", bufs=2))
            wsmall = ctx.enter_context(tc.tile_pool(name="wsmall", bufs=8))
            epool = ctx.enter_context(tc.tile_pool(name="epool", bufs=4))
            pfc = ctx.enter_context(tc.tile_pool(name="pfc", bufs=2, space="PSUM"))
            pg = ctx.enter_context(tc.tile_pool(name="pg", bufs=2, space="PSUM"))
            ps = ctx.enter_context(tc.tile_pool(name="ps", bufs=1, space="PSUM"))
            pc = ctx.enter_context(tc.tile_pool(name="pc", bufs=1, space="PSUM"))

            # ---- prime ACT (act-table load) and warm up PE immediately ----
            ones_bf = singles.tile([128, 128], bf16)
            nc.vector.memset(ones_bf, 1.0)
            prime = singles.tile([128, 8], f32)
            nc.scalar.copy(prime, ones_bf[:, 0:8])
            warm_ps = pfc.tile([128, 512], f32, tag="fc")
            nc.tensor.matmul(warm_ps[:, 0:128], lhsT=ones_bf, rhs=ones_bf)

            # ---- input DMAs. Everything is host-pretransposed, so these
            # are plain copies. The four small critical params issue from
            # the ACT HWDGE queue first (they carry no DMA-engine-reuse
            # waits, so they don't block ACT's compute stream); the bulk
            # sequence tensors stream on the SP queue. ----
            p8t = singles.tile([128, LP], u16)
            r8t = singles.tile([128, NQP], u16)
            wq8 = singles.tile([128, 2, KD], fp8)
            wk8 = singles.tile([128, 2, KD], fp8)
            bcols = singles.tile([128, 14], f32)
            mcols = singles.tile([128, 32 + NQT], f32)

            # first chunks of p8/r8 go on the SWDGE queue so the FC ladder
            # starts ~3.5us; the rest streams behind them
            nc.gpsimd.dma_start(p8t[:, 0:512], p8_in[:, 0:512])
            nc.gpsimd.dma_start(r8t[:, 0 : min(512, NQP)], r8_in[:, 0 : min(512, NQP)])
            nc.gpsimd.dma_start(p8t[:, 512:LP], p8_in[:, 512:LP])
            if NQP > 512:
                nc.gpsimd.dma_start(r8t[:, 512:NQP], r8_in[:, 512:NQP])
            nc.sync.dma_start(wq8, wq8_in[:])
            nc.sync.dma_start(bcols, bias_cols[:])
            nc.sync.dma_start(wk8, wk8_in[:])
            nc.sync.dma_start(mcols, mask_cols[:])
            wk_bf = singles.tile([128, NHC, KD], bf16)
            nc.sync.dma_start(wk_bf, wkbf_in[:])
            rtb = singles.tile([128, NHC, NQP], bf16)
            nc.sync.dma_start(rtb, rbf_in[:])
            kbd_bc = singles.tile([128, KD], bf16)
            nc.gpsimd.dma_start(kbd_bc, kbd_in[:].partition_broadcast(128))

            qb_col = bcols[:, 0:NKC]                 # signed 64*|h|*Qb (or 64*Qb mix)
            kb64_col = bcols[:, NKC : 2 * NKC]       # 64*Kb
            h_col = bcols[:, 2 * NKC : 3 * NKC]      # h (mix chunks only)
            ident2 = bcols[:, 12:14]                 # eye(2) on partitions 0-1
            mp_col = mcols[:, 0:NLT]
            valid_col = mcols[:, NLT : 2 * NLT]
            mv_col = mcols[:, 2 * NLT :]             # mask_v/(64L), packed

            # reduction stationary, DoubleRow-paired over u-tile pairs (Pool
            # builds it so DVE stays clear for the FC evacuations)
            rbuf8 = singles.tile([128, 2, NLT // 2, 2], fp8)
            for ko in range(2):
                nc.gpsimd.tensor_copy(rbuf8[:, ko, :, 0], valid_col[:, ko::2])
                nc.gpsimd.tensor_copy(rbuf8[:, ko, :, 1], mp_col[:, ko::2])

            # fp8 views with the h-pair as the DoubleRow interleave dim
            p8v = p8t[:].bitcast(fp8).rearrange("p (l two) -> p two l", two=2)
            r8v = r8t[:].bitcast(fp8).rearrange("p (l two) -> p two l", two=2)

            ut8 = singles.tile([128, NKC, LP], fp8)
            vt8 = singles.tile([128, NKC, NQP], fp8)
            vnat = singles.tile([128, NQT, KD], f32r)
            wcol = singles.tile([128, NQT], f32r)
            s_sb = singles.tile([2, NQP], f32)

            # ---- FC phase: ut (Q-side of P, all 2048 u) and vt (K-side of
            # R, valid q only), fp8 DoubleRow, evacuated straight to fp8.
            # vt chunks >=2 are deferred into G-loop slots so the DVE
            # stream reaches the span-0 S evacuation without backlog. ----
            def ut_fc(vc, kc):
                sl = slice(vc * 512, (vc + 1) * 512)
                pm = pfc.tile([128, 512], f32, tag="fc")
                nc.tensor.matmul(
                    pm,
                    lhsT=wq8[:, :, kc * 128 : (kc + 1) * 128],
                    rhs=p8v[:, :, sl],
                    perf_mode=DR,
                )
                if modes[kc] == "mix":
                    sb = stage.tile([128, 512], bf16, tag="st")
                    nc.vector.tensor_scalar(
                        sb, pm, qb_col[:, kc : kc + 1], 0.0, ALU.add, ALU.max
                    )
                    nc.gpsimd.tensor_scalar_mul(
                        ut8[:, kc, sl], sb, h_col[:, kc : kc + 1]
                    )
                else:
                    op1 = ALU.max if modes[kc] == "max" else ALU.min
                    nc.vector.tensor_scalar(
                        ut8[:, kc, sl], pm, qb_col[:, kc : kc + 1], 0.0,
                        ALU.add, op1,
                    )

            def vt_fc(vc, kc, use_pg=False):
                qs = slice(vc * 512, min((vc + 1) * 512, NQP))
                if use_pg:
                    # pg is free during the FC phase: running the inline vt
                    # ladder there decouples it from the ut ladder's pfc
                    # buffer rotation (both evac chains then overlap)
                    pmt = pg.tile([128, 2, 512], f32, tag="g", name=f"vt_ps_{vc}_{kc}")
                    pm2 = pmt[:, 0, :]
                else:
                    pm2 = pfc.tile([128, 512], f32, tag="fc")
                nc.tensor.matmul(
                    pm2[:, 0 : qs.stop - qs.start],
                    lhsT=wk8[:, :, kc * 128 : (kc + 1) * 128],
                    rhs=r8v[:, :, qs],
                    perf_mode=DR,
                )
                if vc == 0:
                    # ACT is otherwise idle before the first exp
                    nc.scalar.activation(
                        vt8[:, kc, qs], pm2[:, 0 : qs.stop - qs.start],
                        AF.Relu, bias=kb64_col[:, kc : kc + 1],
                    )
                else:
                    nc.vector.tensor_scalar(
                        vt8[:, kc, qs], pm2[:, 0 : qs.stop - qs.start],
                        kb64_col[:, kc : kc + 1], 0.0, ALU.add, ALU.max,
                    )

            nqc = (NQP + 511) // 512  # vt chunks
            inline_vt = 1 if NSP >= 2 else nqc
            # interleave the ut and inline-vt ladders (separate psum pools)
            for kc in range(NKC):
                ut_fc(0, kc)
                for vc in range(min(inline_vt, nqc)):
                    vt_fc(vc, kc, use_pg=True)
            for vc in range(1, 4):
                for kc in range(NKC):
                    ut_fc(vc, kc)
            # deferred vt chunk c runs in span c-1's slots (DVE reaches it
            # right after the ut chain, before the span's S evacuation)
            vt_sched = {}
            for c in range(inline_vt, nqc):
                vt_sched.setdefault(min(c - 1, NSP - 1), []).append(c)

            # ---- value chain (bf16), w math, contrib: interleaved into the
            # G loop's slots below ----
            # value chain: same fp8 DoubleRow operands as the vt FC, but
            # transposed layout ([q, k]); bias rides a bf16 ones x Kb/128
            # matmul; the 64x scale is folded into the w column scale
            def fc_nat(qt):
                pm = pfc.tile([128, 512], f32, tag="fc")
                for hc in range(NHC):
                    nc.tensor.matmul(
                        pm,
                        lhsT=rtb[:, hc, qt * 128 : (qt + 1) * 128],
                        rhs=wk_bf[:, hc, :],
                        start=(hc == 0),
                        stop=False,
                    )
                nc.tensor.matmul(
                    pm, lhsT=ones_bf, rhs=kbd_bc[:],
                    start=False, stop=True, skip_group_check=True,
                )
                nc.vector.tensor_scalar_max(vnat[:, qt, :], pm, 0.0)

            def w_math(si):
                q0, w = spans[si]
                nc.vector.tensor_copy(s_sb[:, q0 : q0 + w], s_t[si][:, 0:w])
                for j in range(w // 128):
                    qt = q0 // 128 + j
                    # s_t[si] is dead once s_sb is copied; reuse its bank so
                    # these tiny transposes stay off the fc_nat pool
                    st_ps = ps.tile([128, 2], f32, tag="s")
                    nc.tensor.transpose(
                        st_ps[:, 0:2],
                        s_sb[:, qt * 128 : (qt + 1) * 128],
                        ident2[0:2, :],
                    )
                    s2 = wsmall.tile([128, 2], f32, tag="s2")
                    nc.vector.tensor_copy(s2, st_ps[:, 0:2])
                    rcp = wsmall.tile([128, 1], f32, tag="rcp")
                    nc.vector.reciprocal(rcp, s2[:, 0:1])
                    nc.vector.tensor_mul(rcp, rcp, s2[:, 1:2])
                    nc.vector.tensor_mul(
                        wcol[:, qt : qt + 1], rcp, mv_col[:, qt : qt + 1]
                    )

            c_ps = pc.tile([1, KD], f32)
            contrib_n = [0]

            def contrib(qt):
                contrib_n[0] += 1
                nc.tensor.matmul(
                    c_ps,
                    lhsT=wcol[:, qt : qt + 1],
                    rhs=vnat[:, qt, :],
                    start=(contrib_n[0] == 1),
                    stop=(contrib_n[0] == NQT),
                    skip_group_check=True,
                )

            # slot scheduler: fc_nat goes into span>=1 slots; a span's
            # contrib mms are emitted once its w_math ran and its fc_nats
            # are all emitted
            fc_done = [False] * NQT
            wm_done = [False] * NSP
            ct_done = [False] * NSP
            fc_iter = iter(range(NQT))

            def try_contrib():
                for si2, (q0, w) in enumerate(spans):
                    if ct_done[si2] or not wm_done[si2]:
                        continue
                    qts = range(q0 // 128, (q0 + w) // 128)
                    if all(fc_done[qt] for qt in qts):
                        for qt in qts:
                            contrib(qt)
                        ct_done[si2] = True

            # fc_nat spreads over spans >=1 (half rate in span 1, where DVE
            # still carries the deferred vt evacuations)
            def slot(si, ltp):
                pend = vt_sched.get(si)
                if pend:
                    c = pend.pop(0)
                    for kc in range(NKC):
                        vt_fc(c, kc)
                    return
                if si >= 1 and not (si == 1 and ltp % 2 == 0):
                    qt = next(fc_iter, None)
                    if qt is not None:
                        fc_nat(qt)
                        fc_done[qt] = True
                        try_contrib()

            # ---- G + exp + S reduction ----
            s_t = [ps.tile([2, 512], f32, tag="s", name=f"s_ps_{si}") for si in range(NSP)]
            for si, (q0, w) in enumerate(spans):
                for ltp in range(NLT // 2):
                    et = epool.tile([128, 2, 512], fp8, tag="e")
                    gp = pg.tile([128, 2, 512], f32, tag="g")
                    for sub in range(2):
                        lt = 2 * ltp + sub
                        for j in range(2):
                            nc.tensor.matmul(
                                gp[:, sub, 0:w],
                                lhsT=ut8[:, 2 * j : 2 * j + 2, lt * 128 : (lt + 1) * 128],
                                rhs=vt8[:, 2 * j : 2 * j + 2, q0 : q0 + w],
                                start=(j == 0),
                                stop=(j == 1),
                                perf_mode=DR,
                            )
                    nc.scalar.activation(
                        et[:, :, 0:w], gp[:, :, 0:w], AF.Exp,
                        scale=1.0 / (WSCALE * WSCALE),
                    )
                    nc.tensor.matmul(
                        s_t[si][:, 0:w],
                        lhsT=rbuf8[:, :, ltp, :],
                        rhs=et[:, :, 0:w],
                        start=(ltp == 0), stop=(ltp == NLT // 2 - 1),
                        perf_mode=DR,
                        skip_group_check=True,
                    )
                    slot(si, ltp)
                w_math(si)
                wm_done[si] = True
                try_contrib()

            # drain any stragglers (small NQT configs)
            for qt in fc_iter:
                fc_nat(qt)
                fc_done[qt] = True
            try_contrib()
            assert all(ct_done), "contrib scheduling incomplete"

            out_sb = singles.tile([1, KD], f32)
            nc.vector.tensor_copy(out_sb, c_ps)
            nc.sync.dma_start(out[:], out_sb)

    nc.finalize()
    return nc


def _get_nc(nqt=13, modes=("max", "max", "min", "min")):
    key = (nqt, tuple(modes))
    if key not in _NC_CACHE:
        _NC_CACHE[key] = _build_nc(nqt, tuple(modes))
    return _NC_CACHE[key]


def kernel(**inputs) -> np.ndarray:
    import ml_dtypes
    from concourse.bass_utils import run_bass_kernel_spmd

    X = np.asarray(inputs["X"], dtype=np.float32)
    Y = np.asarray(inputs["Y"], dtype=np.float32)
    m1 = np.asarray(inputs["mask1"], dtype=np.float32)
    m2 = np.asarray(inputs["mask2"], dtype=np.float32)
    Qv = np.asarray(inputs["Qv"], dtype=np.float32)
    Qg = np.float32(np.asarray(inputs["Qg"]))
    Qb = np.asarray(inputs["Qb"], dtype=np.float32)
    Kv = np.asarray(inputs["Kv"], dtype=np.float32)
    Kg = np.float32(np.asarray(inputs["Kg"]))
    Kb = np.asarray(inputs["Kb"], dtype=np.float32)
    hm = np.asarray(inputs["h_mat"], dtype=np.float32)
    gamma = np.asarray(inputs["gamma"], dtype=np.float32)
    beta = np.asarray(inputs["beta"], dtype=np.float32)

    Wq = (Qg / np.float32(np.linalg.norm(Qv))) * Qv  # [KD, HD]
    Wk = (Kg / np.float32(np.linalg.norm(Kv))) * Kv

    # permute k so sign(h) is constant per 128-chunk where possible
    perm_k = np.argsort(hm < 0, kind="stable")
    hmp = hm[perm_k]
    Wqp, Qbp = Wq[perm_k], Qb[perm_k]
    Wkp, Kbp = Wk[perm_k], Kb[perm_k]
    modes = []
    wq_rows = np.empty_like(Wqp)
    qb_eff = np.empty_like(Qbp)
    for kc in range(4):
        slk = slice(kc * 128, (kc + 1) * 128)
        hs = hmp[slk]
        if np.all(hs >= 0):
            modes.append("max")
            sc = hs
        elif np.all(hs < 0):
            modes.append("min")
            sc = hs  # negative scale = |h| fold + weight negation in one
        else:
            modes.append("mix")
            sc = np.ones_like(hs)
        wq_rows[slk] = sc[:, None] * Wqp[slk]
        qb_eff[slk] = sc * Qbp[slk]
    modes = tuple(modes)

    # 64*W^T with rows pairing consecutive h for the DoubleRow interleave
    wq8_in = np.ascontiguousarray(
        (WSCALE * wq_rows.T).reshape(128, 2, KD).astype(ml_dtypes.float8_e4m3)
    )
    wk8_in = np.ascontiguousarray(
        (WSCALE * Wkp.T).reshape(128, 2, KD).astype(ml_dtypes.float8_e4m3)
    )
    ey = np.zeros((128, 2), np.float32)
    ey[0, 0] = ey[1, 1] = 1.0
    bias_cols = np.ascontiguousarray(
        np.concatenate(
            [
                (WSCALE * qb_eff).reshape(4, 128).T,
                (WSCALE * Kbp).reshape(4, 128).T,
                hmp.reshape(4, 128).T,
                ey,
            ],
            axis=1,
        )
    ).astype(np.float32)  # [128, 14]
    wkbf_in = np.ascontiguousarray(
        (WSCALE * Wkp.T).reshape(2, 128, KD).transpose(1, 0, 2)
    ).astype(ml_dtypes.bfloat16)
    kbd = (WSCALE * Kbp / 128.0).astype(ml_dtypes.bfloat16)

    def padded(v2000):
        p = np.zeros((LP,), np.float32)
        p[:L] = v2000
        return p.reshape(16, 128)

    valid = padded(np.ones(L, np.float32))

    def pad_seq(s):
        p = np.zeros((LP, HD), np.float32)
        p[:L] = s
        return p

    # Only q columns with mask_v > 0 contribute; permute them to the front
    # and trim the computed q window to 128-col tiles covering the max
    # valid count across cores.
    units = []
    max_nv = 0
    for b in range(B):
        for m in range(2):
            if m == 0:
                P, R, mp, mv = X[b], Y[b], m1[b], m2[b]
            else:
                P, R, mp, mv = Y[b], X[b], m2[b], m1[b]
            perm = np.argsort(mv <= 0, kind="stable")
            max_nv = max(max_nv, int((mv > 0).sum()))
            units.append((P, R, mp, mv, perm))
    nqt = max(1, -(-max_nv // 128))
    NQP = 128 * nqt

    in_maps = []
    for P, R, mp, mv, perm in units:
        nperm = min(NQP, L)
        Rp = np.zeros((NQP, HD), np.float32)
        Rp[:nperm] = R[perm[:nperm]]
        mvp = np.zeros((NQP,), np.float32)
        mvp[:nperm] = mv[perm[:nperm]] * (1.0 / (L * WSCALE))
        mask_cols = np.ascontiguousarray(
            np.concatenate(
                [padded(mp), valid, mvp.reshape(nqt, 128)], axis=0
            ).T
        ).astype(np.float32)  # [128, 32 + nqt]
        p8 = np.ascontiguousarray(
            pad_seq(P).astype(ml_dtypes.float8_e4m3).view(np.uint16).T
        )
        r8 = np.ascontiguousarray(
            Rp.astype(ml_dtypes.float8_e4m3).view(np.uint16).T
        )
        rbf = np.ascontiguousarray(
            Rp.astype(ml_dtypes.bfloat16).T.reshape(2, 128, NQP).transpose(1, 0, 2)
        )
        in_maps.append(
            {
                "p8_in": p8,
                "r8_in": r8,
                "rbf_in": rbf,
                "wq8_in": wq8_in,
                "wk8_in": wk8_in,
                "bias_cols": bias_cols,
                "mask_cols": mask_cols,
                "wkbf_in": wkbf_in,
                "kbd_in": kbd,
            }
        )

    nc = _get_nc(nqt, modes)
    res = run_bass_kernel_spmd(nc, in_maps, core_ids=list(range(NCORES)))
    contribs = np.stack([r["out"][0] for r in res.results]).astype(np.float64)

    pooled = contribs[0::2] + contribs[1::2]  # [B, KD] in permuted-k order
    mu = pooled.mean(axis=0)
    var = pooled.var(axis=0)
    outv = gamma[perm_k] * (pooled - mu) / np.sqrt(var + EPS) + beta[perm_k]
    inv = np.empty_like(perm_k)
    inv[perm_k] = np.arange(KD)
    return outv[:, inv].astype(np.float32)
